# revision 7
# baseline (speedup 1.0000x reference)
"""Cross-attention Trainium2 kernel (nn_CrossAttention).

Reference computation (per batch b):
    img_n = LN(image_features) ; txt_n = LN(text_features)
    q = img_n @ Wq + bq ; k = txt_n @ Wk + bk
    v = txt_n @ Wv + bv ; v_img = img_n @ Wv + bv
    P = softmax(q @ k^T / sqrt(D))
    image_out = P @ v ; text_out = P^T @ v_img

Sharding: data-parallel over batch: 16 batches -> 8 cores x 2 batches.
No collectives; each core computes its batches end to end.

Device algorithm per core (everything fp16 operands, fp32 PSUM accum):
  - Host folds ln_g into the weights and combines the S path:
        W_qk = (g*Wk) @ (g*Wq)^T   so   S = img_n0 @ (txt_n0 @ W_qk)^T
    which projects only the (smaller) text side for the S matmul.
  - text_out is reassociated: text_out = (P^T @ img_n) @ Wv, so image
    tokens are never projected through Wv.
  - LN is computed in natural layout (bn_stats), normalized output is
    cast to fp16 and transposed on the tensor engine (via identity) to
    get d-on-partition layouts for the projections.
  - exp(S/sqrt(D)) is computed with no max subtraction (|S|/32 < 2 for
    LN'd inputs by construction) during PSUM eviction on ScalarE, with
    row sums accumulated for free via accum_out.
"""

import numpy as np

N_CORES = 8
D = 1024
DT = D // 128          # 8 d-tiles
EPS = 1e-5

F16 = "float16"
F32 = "float32"


def _build_program(BL, Ni, Nt):
    """Build the per-core bass program. BL batches, Ni image tokens,
    Nt text tokens per batch; returns compiled Bacc."""
    import concourse.bass as bass
    import concourse.tile as tile
    from concourse import bacc, mybir
    from concourse.masks import make_identity

    f16 = mybir.dt.float16
    f32 = mybir.dt.float32
    Exp = mybir.ActivationFunctionType.Exp
    Log = mybir.ActivationFunctionType.Ln
    Copy = mybir.ActivationFunctionType.Copy

    IT = Ni // 128          # 16 image tiles / batch
    TT = Nt // 128          # 8 text tiles / batch
    NCH = Ni // 512         # 4 image chunks / batch
    JC = 512 // 128         # 4 i-tiles per chunk
    scale = float(D) ** -0.5

    nc = bacc.Bacc("TRN2", target_bir_lowering=False, debug=False,
                   num_devices=N_CORES)

    img_d = nc.dram_tensor("img", [BL, Ni, D], f32, kind="ExternalInput").ap()
    txt_d = nc.dram_tensor("txt", [BL, Nt, D], f32, kind="ExternalInput").ap()
    wqk_d = nc.dram_tensor("wqk", [D, D], f16, kind="ExternalInput").ap()
    wv_d = nc.dram_tensor("wv", [D, D], f16, kind="ExternalInput").ap()
    oimg_d = nc.dram_tensor("oimg", [BL, Ni, D], f32, kind="ExternalOutput").ap()
    otxt_d = nc.dram_tensor("otxt", [BL, Nt, D], f32, kind="ExternalOutput").ap()

    with tile.TileContext(nc) as tc:
        from contextlib import ExitStack
        with ExitStack() as ctx:
            def pool(name, bufs, space="SBUF"):
                return ctx.enter_context(
                    tc.tile_pool(name=name, bufs=bufs, space=space))

            p_const = pool("const", 1)
            p_w = pool("w", 2)            # weight tiles, 2 MB each
            p_xtt = pool("xtt", 1)        # txt_n^T            2 MB
            p_kpt = pool("kpt", 1)        # k'^T               2 MB
            p_v = pool("v", 1)            # v natural          2 MB
            p_xti = pool("xti", 2)        # img_n^T chunk      1 MB x2
            p_x16c = pool("x16c", 2)      # img_n natural chnk 1 MB x2
            p_es = pool("es", 1)          # expS / P chunk     1 MB
            p_est = pool("est", 2)        # P^T chunk / T1_16  1 MB x2
            p_t1 = pool("t1", 1)          # T1 accumulator f32 4 MB
            p_xn = pool("xn", 2)          # f32 input staging  .5 MB x2
            p_x16t = pool("x16t", 2)      # txt x16 transient  .25 MB x2
            p_ost = pool("ost", 2)        # f32 out staging    .5 MB x2
            p_stat = pool("stat", 4)      # small stats
            p_mm = pool("mm", 5, space="PSUM")          # PSUM matmul tiles
            p_tp = pool("tp", 2, space="PSUM")          # PSUM transpose tiles

            ident = p_const.tile([128, 128], f16, tag="ident")
            make_identity(nc, ident)
            eps_t = p_const.tile([128, 1], f32, tag="eps")
            nc.vector.memset(eps_t, EPS)

            def mmtile():
                return p_mm.tile([128, 512], f32, tag="mm", name="mm")

            def ln_transpose(src_ap, xt_dst_cols, nat_out=None):
                """Load a [128, D] f32 tile, layer-norm it, cast to fp16 and
                write its transpose into xt_dst_cols ([128, DT, 128] view of
                the d-on-partition tensor). Optionally keep the natural fp16
                tile in nat_out ([128, D])."""
                xn = p_xn.tile([128, D], f32, tag="xn")
                nc.sync.dma_start(out=xn, in_=src_ap)
                st = p_stat.tile([128, 2, 6], f32, tag="lnst")
                nc.vector.bn_stats(st[:, 0, :], xn[:, 0:512])
                nc.vector.bn_stats(st[:, 1, :], xn[:, 512:1024])
                mv = p_stat.tile([128, 2], f32, tag="lnmv")
                nc.vector.bn_aggr(mv, st)
                # rstd = exp(-0.5 * log(var + eps)); Log+Exp share one ACT
                # table set (natural_log_exp) unlike Sqrt.
                lnv = p_stat.tile([128, 1], f32, tag="lnv")
                nc.scalar.activation(lnv, mv[:, 1:2], Log, bias=eps_t)
                rstd = p_stat.tile([128, 1], f32, tag="rstd")
                nc.scalar.activation(rstd, lnv, Exp, scale=-0.5)
                if nat_out is None:
                    x16 = p_x16t.tile([128, D], f16, tag="x16t")
                else:
                    x16 = nat_out
                nc.vector.tensor_scalar(
                    out=x16, in0=xn, scalar1=mv[:, 0:1], scalar2=rstd,
                    op0=mybir.AluOpType.subtract, op1=mybir.AluOpType.mult)
                for dg in range(2):
                    pst = p_tp.tile([128, 512], f16, tag="tp", name="tp")
                    for k in range(4):
                        dt = dg * 4 + k
                        nc.tensor.transpose(
                            pst[:, k * 128:(k + 1) * 128],
                            x16[:, dt * 128:(dt + 1) * 128], ident)
                    nc.scalar.activation(
                        xt_dst_cols[:, dg * 4:dg * 4 + 4, :],
                        pst.rearrange("p (a b) -> p a b", a=4), Copy)

            for b in range(BL):
                # ---- weights for this batch ----
                wqk = p_w.tile([128, DT, D], f16, tag="w")
                nc.sync.dma_start(out=wqk, in_=wqk_d.rearrange("(a p) n -> p a n", p=128))
                wv = p_w.tile([128, DT, D], f16, tag="w")
                nc.sync.dma_start(out=wv, in_=wv_d.rearrange("(a p) n -> p a n", p=128))

                # ---- text side: LN+transpose, k'^T and v projections ----
                xtt = p_xtt.tile([128, DT, Nt], f16, tag="xtt")
                for tt in range(TT):
                    ln_transpose(txt_d[b, tt * 128:(tt + 1) * 128, :],
                                 xtt[:, :, tt * 128:(tt + 1) * 128])

                kpt = p_kpt.tile([128, DT, Nt], f16, tag="kpt")
                for p in range(DT):
                    ps0, ps1 = mmtile(), mmtile()
                    for dt in range(DT):
                        lw = wqk[:, dt, p * 128:(p + 1) * 128]
                        nc.tensor.matmul(ps0, lw, xtt[:, dt, 0:512],
                                         start=dt == 0, stop=dt == DT - 1)
                        nc.tensor.matmul(ps1, lw, xtt[:, dt, 512:1024],
                                         start=dt == 0, stop=dt == DT - 1)
                    nc.scalar.activation(kpt[:, p, 0:512], ps0, Copy)
                    nc.scalar.activation(kpt[:, p, 512:1024], ps1, Copy)

                v = p_v.tile([128, TT, D], f16, tag="v")
                for tt in range(TT):
                    ps0, ps1 = mmtile(), mmtile()
                    for dt in range(DT):
                        lw = xtt[:, dt, tt * 128:(tt + 1) * 128]
                        nc.tensor.matmul(ps0, lw, wv[:, dt, 0:512],
                                         start=dt == 0, stop=dt == DT - 1)
                        nc.tensor.matmul(ps1, lw, wv[:, dt, 512:1024],
                                         start=dt == 0, stop=dt == DT - 1)
                    nc.scalar.activation(v[:, tt, 0:512], ps0, Copy)
                    nc.scalar.activation(v[:, tt, 512:1024], ps1, Copy)

                t1 = p_t1.tile([128, DT, Nt], f32, tag="t1")

                # ---- image chunks ----
                for c in range(NCH):
                    xti = p_xti.tile([128, DT, 512], f16, tag="xti")
                    x16c = p_x16c.tile([128, JC, D], f16, tag="x16c")
                    for j in range(JC):
                        it = c * JC + j
                        ln_transpose(img_d[b, it * 128:(it + 1) * 128, :],
                                     xti[:, :, j * 128:(j + 1) * 128],
                                     nat_out=x16c[:, j, :])

                    # S natural + exp + row sums + normalize (in place)
                    es = p_es.tile([128, JC, Nt], f16, tag="es")
                    sums = p_stat.tile([128, JC, 2], f32, tag="sums")
                    inv = p_stat.tile([128, JC], f32, tag="inv")
                    for j in range(JC):
                        ps0, ps1 = mmtile(), mmtile()
                        for dt in range(DT):
                            lw = xti[:, dt, j * 128:(j + 1) * 128]
                            nc.tensor.matmul(ps0, lw, kpt[:, dt, 0:512],
                                             start=dt == 0, stop=dt == DT - 1)
                            nc.tensor.matmul(ps1, lw, kpt[:, dt, 512:1024],
                                             start=dt == 0, stop=dt == DT - 1)
                        nc.scalar.activation(es[:, j, 0:512], ps0, Exp,
                                             scale=scale,
                                             accum_out=sums[:, j, 0:1])
                        nc.scalar.activation(es[:, j, 512:1024], ps1, Exp,
                                             scale=scale,
                                             accum_out=sums[:, j, 1:2])
                        tot = p_stat.tile([128, 1], f32, tag="tot")
                        nc.vector.tensor_reduce(tot, sums[:, j, :],
                                                axis=mybir.AxisListType.X,
                                                op=mybir.AluOpType.add)
                        nc.vector.reciprocal(inv[:, j:j + 1], tot)
                        nc.vector.tensor_scalar_mul(es[:, j, :], es[:, j, :],
                                                    inv[:, j:j + 1])

                    # P^T via tensor-engine transposes
                    est = p_est.tile([128, TT, 512], f16, tag="est")
                    for tt in range(TT):
                        pst = p_tp.tile([128, 512], f16, tag="tp", name="tp")
                        for j in range(JC):
                            nc.tensor.transpose(
                                pst[:, j * 128:(j + 1) * 128],
                                es[:, j, tt * 128:(tt + 1) * 128], ident)
                        nc.scalar.activation(est[:, tt, :], pst, Copy)

                    # image_out rows for this chunk
                    for j in range(JC):
                        ps0, ps1 = mmtile(), mmtile()
                        for tt in range(TT):
                            lw = est[:, tt, j * 128:(j + 1) * 128]
                            nc.tensor.matmul(ps0, lw, v[:, tt, 0:512],
                                             start=tt == 0, stop=tt == TT - 1)
                            nc.tensor.matmul(ps1, lw, v[:, tt, 512:1024],
                                             start=tt == 0, stop=tt == TT - 1)
                        ost = p_ost.tile([128, D], f32, tag="ost")
                        nc.vector.tensor_copy(ost[:, 0:512], ps0)
                        nc.vector.tensor_copy(ost[:, 512:1024], ps1)
                        it = c * JC + j
                        nc.sync.dma_start(
                            out=oimg_d[b, it * 128:(it + 1) * 128, :], in_=ost)

                    # T1^T partial accumulation: T1^T[d,t] += img_n^T P
                    for dt in range(DT):
                        ps0, ps1 = mmtile(), mmtile()
                        for j in range(JC):
                            lw = x16c[:, j, dt * 128:(dt + 1) * 128]
                            nc.tensor.matmul(ps0, lw, es[:, j, 0:512],
                                             start=j == 0, stop=j == JC - 1)
                            nc.tensor.matmul(ps1, lw, es[:, j, 512:1024],
                                             start=j == 0, stop=j == JC - 1)
                        if c == 0:
                            nc.vector.tensor_copy(t1[:, dt, 0:512], ps0)
                            nc.vector.tensor_copy(t1[:, dt, 512:1024], ps1)
                        else:
                            nc.vector.tensor_add(t1[:, dt, 0:512],
                                                 t1[:, dt, 0:512], ps0)
                            nc.vector.tensor_add(t1[:, dt, 512:1024],
                                                 t1[:, dt, 512:1024], ps1)

                # ---- text_out = T1 @ Wv ----
                t16a = p_est.tile([128, JC, Nt], f16, tag="est")
                t16b = p_est.tile([128, JC, Nt], f16, tag="est")
                for dt in range(DT):
                    dst = t16a if dt < JC else t16b
                    nc.vector.tensor_copy(dst[:, dt % JC, :], t1[:, dt, :])
                for tt in range(TT):
                    ps0, ps1 = mmtile(), mmtile()
                    for dt in range(DT):
                        src = t16a if dt < JC else t16b
                        lw = src[:, dt % JC, tt * 128:(tt + 1) * 128]
                        nc.tensor.matmul(ps0, lw, wv[:, dt, 0:512],
                                         start=dt == 0, stop=dt == DT - 1)
                        nc.tensor.matmul(ps1, lw, wv[:, dt, 512:1024],
                                         start=dt == 0, stop=dt == DT - 1)
                    ost = p_ost.tile([128, D], f32, tag="ost")
                    nc.scalar.activation(ost[:, 0:512], ps0, Copy)
                    nc.scalar.activation(ost[:, 512:1024], ps1, Copy)
                    nc.sync.dma_start(
                        out=otxt_d[b, tt * 128:(tt + 1) * 128, :], in_=ost)

    nc.compile()
    return nc


def _prep_host(image_features, text_features, ln_g, ln_b, Wq, bq, Wk, bk, Wv, bv):
    """Host-side weight folding. Returns (wqk16, wv16)."""
    g = np.asarray(ln_g, np.float64)
    b = np.asarray(ln_b, np.float64)
    Wq = np.asarray(Wq, np.float64)
    Wk = np.asarray(Wk, np.float64)
    Wv = np.asarray(Wv, np.float64)
    rows = [b @ Wq + np.asarray(bq, np.float64),
            b @ Wk + np.asarray(bk, np.float64),
            b @ Wv + np.asarray(bv, np.float64)]
    if any(np.abs(r).max() > 0 for r in rows):
        raise NotImplementedError(
            "nonzero LN shift / projection biases not supported by this kernel")
    Wq_e = g[:, None] * Wq
    Wk_e = g[:, None] * Wk
    Wv_e = g[:, None] * Wv
    wqk = (Wk_e @ Wq_e.T).astype(np.float32).astype(np.float16)
    wv = Wv_e.astype(np.float32).astype(np.float16)
    return np.ascontiguousarray(wqk), np.ascontiguousarray(wv)


_PROGRAM_CACHE = {}


def _get_program(BL, Ni, Nt):
    key = (BL, Ni, Nt)
    if key not in _PROGRAM_CACHE:
        _PROGRAM_CACHE[key] = _build_program(BL, Ni, Nt)
    return _PROGRAM_CACHE[key]


def run(image_features, text_features, ln_g, ln_b, Wq, bq, Wk, bk, Wv, bv,
        trace=False, trace_kwargs=None):
    from concourse.bass_utils import run_bass_kernel_spmd

    image_features = np.asarray(image_features, np.float32)
    text_features = np.asarray(text_features, np.float32)
    B, Ni, d = image_features.shape
    _, Nt, _ = text_features.shape
    assert d == D and B % N_CORES == 0
    BL = B // N_CORES

    wqk16, wv16 = _prep_host(image_features, text_features, ln_g, ln_b,
                             Wq, bq, Wk, bk, Wv, bv)
    nc = _get_program(BL, Ni, Nt)

    in_maps = []
    for c in range(N_CORES):
        in_maps.append({
            "img": np.ascontiguousarray(image_features[c * BL:(c + 1) * BL]),
            "txt": np.ascontiguousarray(text_features[c * BL:(c + 1) * BL]),
            "wqk": wqk16,
            "wv": wv16,
        })
    res = run_bass_kernel_spmd(nc, in_maps, core_ids=list(range(N_CORES)),
                               trace=trace, **(trace_kwargs or {}))
    img_out = np.concatenate([res.results[c]["oimg"] for c in range(N_CORES)], axis=0)
    txt_out = np.concatenate([res.results[c]["otxt"] for c in range(N_CORES)], axis=0)
    return (img_out, txt_out), res


def kernel(image_features, text_features, ln_g, ln_b, Wq, bq, Wk, bk, Wv, bv):
    (img_out, txt_out), _ = run(image_features, text_features, ln_g, ln_b,
                                Wq, bq, Wk, bk, Wv, bv)
    return (img_out, txt_out)


# revision 8
# speedup vs baseline: 1.0729x; 1.0729x over previous
"""Cross-attention Trainium2 kernel (nn_CrossAttention).

Reference computation (per batch b):
    img_n = LN(image_features) ; txt_n = LN(text_features)
    q = img_n @ Wq + bq ; k = txt_n @ Wk + bk
    v = txt_n @ Wv + bv ; v_img = img_n @ Wv + bv
    P = softmax(q @ k^T / sqrt(D))
    image_out = P @ v ; text_out = P^T @ v_img

Sharding: data-parallel over batch: 16 batches -> 8 cores x 2 batches.
No collectives; each core computes its batches end to end.

Device algorithm per core (everything fp16 operands, fp32 PSUM accum):
  - Host folds ln_g into the weights and combines the S path:
        W_qk = (g*Wk) @ (g*Wq)^T   so   S = img_n0 @ (txt_n0 @ W_qk)^T
    which projects only the (smaller) text side for the S matmul.
  - text_out is reassociated: text_out = (P^T @ img_n) @ Wv, so image
    tokens are never projected through Wv.
  - LN is computed in natural layout (bn_stats), normalized output is
    cast to fp16 and transposed on the tensor engine (via identity) to
    get d-on-partition layouts for the projections.
  - exp(S/sqrt(D)) is computed with no max subtraction (|S|/32 < 2 for
    LN'd inputs by construction) during PSUM eviction on ScalarE, with
    row sums accumulated for free via accum_out.
"""

import numpy as np

N_CORES = 8
D = 1024
DT = D // 128          # 8 d-tiles
EPS = 1e-5

F16 = "float16"
F32 = "float32"


def _build_program(BL, Ni, Nt):
    """Build the per-core bass program. BL batches, Ni image tokens,
    Nt text tokens per batch; returns compiled Bacc."""
    import concourse.bass as bass
    import concourse.tile as tile
    from concourse import bacc, mybir
    from concourse.masks import make_identity

    f16 = mybir.dt.float16
    f32 = mybir.dt.float32
    Exp = mybir.ActivationFunctionType.Exp
    Log = mybir.ActivationFunctionType.Ln
    Copy = mybir.ActivationFunctionType.Copy

    IT = Ni // 128          # 16 image tiles / batch
    TT = Nt // 128          # 8 text tiles / batch
    NCH = Ni // 512         # 4 image chunks / batch
    JC = 512 // 128         # 4 i-tiles per chunk
    scale = float(D) ** -0.5

    # The act-table-load inserter greedily picks `exp_and_others` for Exp
    # and `natural_log` for Ln, thrashing ~2.6us of table loads per LN
    # tile. Constrain its choices so both funcs resolve to the combined
    # `natural_log_exp_and_others` set (entry order/indices preserved, so
    # act_func_set_ids still match act_info.json).
    import concourse.bacc as bacc_mod
    import concourse.hw_specs as hw_specs
    _orig_tables = hw_specs.get_activation_tables

    def _pinned_tables(arch):
        A = mybir.ActivationFunctionType
        out = {}
        for name, fns in _orig_tables(arch).items():
            if name != "natural_log_exp_and_others":
                fns = fns - {A.Exp, A.Ln}
            out[name] = fns
        return out

    bacc_mod.get_activation_tables = _pinned_tables

    nc = bacc.Bacc("TRN2", target_bir_lowering=False, debug=False,
                   num_devices=N_CORES)

    img_d = nc.dram_tensor("img", [BL, Ni, D], f32, kind="ExternalInput").ap()
    txt_d = nc.dram_tensor("txt", [BL, Nt, D], f32, kind="ExternalInput").ap()
    wqk_d = nc.dram_tensor("wqk", [D, D], f16, kind="ExternalInput").ap()
    wv_d = nc.dram_tensor("wv", [D, D], f16, kind="ExternalInput").ap()
    oimg_d = nc.dram_tensor("oimg", [BL, Ni, D], f32, kind="ExternalOutput").ap()
    otxt_d = nc.dram_tensor("otxt", [BL, Nt, D], f32, kind="ExternalOutput").ap()

    with tile.TileContext(nc) as tc:
        from contextlib import ExitStack
        with ExitStack() as ctx:
            def pool(name, bufs, space="SBUF"):
                return ctx.enter_context(
                    tc.tile_pool(name=name, bufs=bufs, space=space))

            p_const = pool("const", 1)
            p_w = pool("w", 2)            # weight tiles, 2 MB each
            p_xtt = pool("xtt", 1)        # txt_n^T            2 MB
            p_kpt = pool("kpt", 1)        # k'^T               2 MB
            p_v = pool("v", 1)            # v natural          2 MB
            p_xti = pool("xti", 2)        # img_n^T chunk      1 MB x2
            p_x16c = pool("x16c", 2)      # img_n natural chnk 1 MB x2
            p_es = pool("es", 1)          # expS / P chunk     1 MB
            p_est = pool("est", 2)        # P^T chunk / T1_16  1 MB x2
            p_t1 = pool("t1", 1)          # T1 accumulator f32 4 MB
            p_xn = pool("xn", 2)          # f32 input staging  .5 MB x2
            p_x16t = pool("x16t", 2)      # txt x16 transient  .25 MB x2
            p_ost = pool("ost", 2)        # f32 out staging    .5 MB x2
            p_stat = pool("stat", 4)      # small stats
            p_mm = pool("mm", 5, space="PSUM")          # PSUM matmul tiles
            p_tp = pool("tp", 2, space="PSUM")          # PSUM transpose tiles

            ident = p_const.tile([128, 128], f16, tag="ident")
            make_identity(nc, ident)
            eps_t = p_const.tile([128, 1], f32, tag="eps")
            nc.vector.memset(eps_t, EPS)

            def mmtile():
                return p_mm.tile([128, 512], f32, tag="mm", name="mm")

            def ln_transpose(src_ap, xt_dst_cols, nat_out=None):
                """Load a [128, D] f32 tile, layer-norm it, cast to fp16 and
                write its transpose into xt_dst_cols ([128, DT, 128] view of
                the d-on-partition tensor). Optionally keep the natural fp16
                tile in nat_out ([128, D])."""
                xn = p_xn.tile([128, D], f32, tag="xn")
                nc.sync.dma_start(out=xn, in_=src_ap)
                st = p_stat.tile([128, 2, 6], f32, tag="lnst")
                nc.vector.bn_stats(st[:, 0, :], xn[:, 0:512])
                nc.vector.bn_stats(st[:, 1, :], xn[:, 512:1024])
                mv = p_stat.tile([128, 2], f32, tag="lnmv")
                nc.vector.bn_aggr(mv, st)
                # rstd = exp(-0.5 * log(var + eps)); Log+Exp share one ACT
                # table set (natural_log_exp) unlike Sqrt.
                lnv = p_stat.tile([128, 1], f32, tag="lnv")
                nc.scalar.activation(lnv, mv[:, 1:2], Log, bias=eps_t)
                rstd = p_stat.tile([128, 1], f32, tag="rstd")
                nc.scalar.activation(rstd, lnv, Exp, scale=-0.5)
                if nat_out is None:
                    x16 = p_x16t.tile([128, D], f16, tag="x16t")
                else:
                    x16 = nat_out
                nc.vector.tensor_scalar(
                    out=x16, in0=xn, scalar1=mv[:, 0:1], scalar2=rstd,
                    op0=mybir.AluOpType.subtract, op1=mybir.AluOpType.mult)
                for dg in range(2):
                    pst = p_tp.tile([128, 512], f16, tag="tp", name="tp")
                    for k in range(4):
                        dt = dg * 4 + k
                        nc.tensor.transpose(
                            pst[:, k * 128:(k + 1) * 128],
                            x16[:, dt * 128:(dt + 1) * 128], ident)
                    nc.scalar.activation(
                        xt_dst_cols[:, dg * 4:dg * 4 + 4, :],
                        pst.rearrange("p (a b) -> p a b", a=4), Copy)

            for b in range(BL):
                # ---- weights for this batch ----
                wqk = p_w.tile([128, DT, D], f16, tag="w")
                nc.sync.dma_start(out=wqk, in_=wqk_d.rearrange("(a p) n -> p a n", p=128))
                wv = p_w.tile([128, DT, D], f16, tag="w")
                nc.sync.dma_start(out=wv, in_=wv_d.rearrange("(a p) n -> p a n", p=128))

                # ---- text side: LN+transpose, k'^T and v projections ----
                xtt = p_xtt.tile([128, DT, Nt], f16, tag="xtt")
                for tt in range(TT):
                    ln_transpose(txt_d[b, tt * 128:(tt + 1) * 128, :],
                                 xtt[:, :, tt * 128:(tt + 1) * 128])

                kpt = p_kpt.tile([128, DT, Nt], f16, tag="kpt")
                for p in range(DT):
                    ps0, ps1 = mmtile(), mmtile()
                    for dt in range(DT):
                        lw = wqk[:, dt, p * 128:(p + 1) * 128]
                        nc.tensor.matmul(ps0, lw, xtt[:, dt, 0:512],
                                         start=dt == 0, stop=dt == DT - 1)
                        nc.tensor.matmul(ps1, lw, xtt[:, dt, 512:1024],
                                         start=dt == 0, stop=dt == DT - 1)
                    nc.scalar.activation(kpt[:, p, 0:512], ps0, Copy)
                    nc.scalar.activation(kpt[:, p, 512:1024], ps1, Copy)

                v = p_v.tile([128, TT, D], f16, tag="v")
                for tt in range(TT):
                    ps0, ps1 = mmtile(), mmtile()
                    for dt in range(DT):
                        lw = xtt[:, dt, tt * 128:(tt + 1) * 128]
                        nc.tensor.matmul(ps0, lw, wv[:, dt, 0:512],
                                         start=dt == 0, stop=dt == DT - 1)
                        nc.tensor.matmul(ps1, lw, wv[:, dt, 512:1024],
                                         start=dt == 0, stop=dt == DT - 1)
                    nc.scalar.activation(v[:, tt, 0:512], ps0, Copy)
                    nc.scalar.activation(v[:, tt, 512:1024], ps1, Copy)

                t1 = p_t1.tile([128, DT, Nt], f32, tag="t1")

                # ---- image chunks ----
                for c in range(NCH):
                    xti = p_xti.tile([128, DT, 512], f16, tag="xti")
                    x16c = p_x16c.tile([128, JC, D], f16, tag="x16c")
                    for j in range(JC):
                        it = c * JC + j
                        ln_transpose(img_d[b, it * 128:(it + 1) * 128, :],
                                     xti[:, :, j * 128:(j + 1) * 128],
                                     nat_out=x16c[:, j, :])

                    # S natural + exp + row sums + normalize (in place)
                    es = p_es.tile([128, JC, Nt], f16, tag="es")
                    sums = p_stat.tile([128, JC, 2], f32, tag="sums")
                    inv = p_stat.tile([128, JC], f32, tag="inv")
                    for j in range(JC):
                        ps0, ps1 = mmtile(), mmtile()
                        for dt in range(DT):
                            lw = xti[:, dt, j * 128:(j + 1) * 128]
                            nc.tensor.matmul(ps0, lw, kpt[:, dt, 0:512],
                                             start=dt == 0, stop=dt == DT - 1)
                            nc.tensor.matmul(ps1, lw, kpt[:, dt, 512:1024],
                                             start=dt == 0, stop=dt == DT - 1)
                        nc.scalar.activation(es[:, j, 0:512], ps0, Exp,
                                             scale=scale,
                                             accum_out=sums[:, j, 0:1])
                        nc.scalar.activation(es[:, j, 512:1024], ps1, Exp,
                                             scale=scale,
                                             accum_out=sums[:, j, 1:2])
                        tot = p_stat.tile([128, 1], f32, tag="tot")
                        nc.vector.tensor_reduce(tot, sums[:, j, :],
                                                axis=mybir.AxisListType.X,
                                                op=mybir.AluOpType.add)
                        nc.vector.reciprocal(inv[:, j:j + 1], tot)
                        nc.vector.tensor_scalar_mul(es[:, j, :], es[:, j, :],
                                                    inv[:, j:j + 1])

                    # P^T via tensor-engine transposes
                    est = p_est.tile([128, TT, 512], f16, tag="est")
                    for tt in range(TT):
                        pst = p_tp.tile([128, 512], f16, tag="tp", name="tp")
                        for j in range(JC):
                            nc.tensor.transpose(
                                pst[:, j * 128:(j + 1) * 128],
                                es[:, j, tt * 128:(tt + 1) * 128], ident)
                        nc.scalar.activation(est[:, tt, :], pst, Copy)

                    # image_out rows for this chunk
                    for j in range(JC):
                        ps0, ps1 = mmtile(), mmtile()
                        for tt in range(TT):
                            lw = est[:, tt, j * 128:(j + 1) * 128]
                            nc.tensor.matmul(ps0, lw, v[:, tt, 0:512],
                                             start=tt == 0, stop=tt == TT - 1)
                            nc.tensor.matmul(ps1, lw, v[:, tt, 512:1024],
                                             start=tt == 0, stop=tt == TT - 1)
                        ost = p_ost.tile([128, D], f32, tag="ost")
                        nc.vector.tensor_copy(ost[:, 0:512], ps0)
                        nc.vector.tensor_copy(ost[:, 512:1024], ps1)
                        it = c * JC + j
                        nc.sync.dma_start(
                            out=oimg_d[b, it * 128:(it + 1) * 128, :], in_=ost)

                    # T1^T partial accumulation: T1^T[d,t] += img_n^T P
                    for dt in range(DT):
                        ps0, ps1 = mmtile(), mmtile()
                        for j in range(JC):
                            lw = x16c[:, j, dt * 128:(dt + 1) * 128]
                            nc.tensor.matmul(ps0, lw, es[:, j, 0:512],
                                             start=j == 0, stop=j == JC - 1)
                            nc.tensor.matmul(ps1, lw, es[:, j, 512:1024],
                                             start=j == 0, stop=j == JC - 1)
                        if c == 0:
                            nc.vector.tensor_copy(t1[:, dt, 0:512], ps0)
                            nc.vector.tensor_copy(t1[:, dt, 512:1024], ps1)
                        else:
                            nc.vector.tensor_add(t1[:, dt, 0:512],
                                                 t1[:, dt, 0:512], ps0)
                            nc.vector.tensor_add(t1[:, dt, 512:1024],
                                                 t1[:, dt, 512:1024], ps1)

                # ---- text_out = T1 @ Wv ----
                t16a = p_est.tile([128, JC, Nt], f16, tag="est")
                t16b = p_est.tile([128, JC, Nt], f16, tag="est")
                for dt in range(DT):
                    dst = t16a if dt < JC else t16b
                    nc.vector.tensor_copy(dst[:, dt % JC, :], t1[:, dt, :])
                for tt in range(TT):
                    ps0, ps1 = mmtile(), mmtile()
                    for dt in range(DT):
                        src = t16a if dt < JC else t16b
                        lw = src[:, dt % JC, tt * 128:(tt + 1) * 128]
                        nc.tensor.matmul(ps0, lw, wv[:, dt, 0:512],
                                         start=dt == 0, stop=dt == DT - 1)
                        nc.tensor.matmul(ps1, lw, wv[:, dt, 512:1024],
                                         start=dt == 0, stop=dt == DT - 1)
                    ost = p_ost.tile([128, D], f32, tag="ost")
                    nc.scalar.activation(ost[:, 0:512], ps0, Copy)
                    nc.scalar.activation(ost[:, 512:1024], ps1, Copy)
                    nc.sync.dma_start(
                        out=otxt_d[b, tt * 128:(tt + 1) * 128, :], in_=ost)

    nc.compile()
    return nc


def _prep_host(image_features, text_features, ln_g, ln_b, Wq, bq, Wk, bk, Wv, bv):
    """Host-side weight folding. Returns (wqk16, wv16)."""
    g = np.asarray(ln_g, np.float64)
    b = np.asarray(ln_b, np.float64)
    Wq = np.asarray(Wq, np.float64)
    Wk = np.asarray(Wk, np.float64)
    Wv = np.asarray(Wv, np.float64)
    rows = [b @ Wq + np.asarray(bq, np.float64),
            b @ Wk + np.asarray(bk, np.float64),
            b @ Wv + np.asarray(bv, np.float64)]
    if any(np.abs(r).max() > 0 for r in rows):
        raise NotImplementedError(
            "nonzero LN shift / projection biases not supported by this kernel")
    Wq_e = g[:, None] * Wq
    Wk_e = g[:, None] * Wk
    Wv_e = g[:, None] * Wv
    wqk = (Wk_e @ Wq_e.T).astype(np.float32).astype(np.float16)
    wv = Wv_e.astype(np.float32).astype(np.float16)
    return np.ascontiguousarray(wqk), np.ascontiguousarray(wv)


_PROGRAM_CACHE = {}


def _get_program(BL, Ni, Nt):
    key = (BL, Ni, Nt)
    if key not in _PROGRAM_CACHE:
        _PROGRAM_CACHE[key] = _build_program(BL, Ni, Nt)
    return _PROGRAM_CACHE[key]


def run(image_features, text_features, ln_g, ln_b, Wq, bq, Wk, bk, Wv, bv,
        trace=False, trace_kwargs=None):
    from concourse.bass_utils import run_bass_kernel_spmd

    image_features = np.asarray(image_features, np.float32)
    text_features = np.asarray(text_features, np.float32)
    B, Ni, d = image_features.shape
    _, Nt, _ = text_features.shape
    assert d == D and B % N_CORES == 0
    BL = B // N_CORES

    wqk16, wv16 = _prep_host(image_features, text_features, ln_g, ln_b,
                             Wq, bq, Wk, bk, Wv, bv)
    nc = _get_program(BL, Ni, Nt)

    in_maps = []
    for c in range(N_CORES):
        in_maps.append({
            "img": np.ascontiguousarray(image_features[c * BL:(c + 1) * BL]),
            "txt": np.ascontiguousarray(text_features[c * BL:(c + 1) * BL]),
            "wqk": wqk16,
            "wv": wv16,
        })
    res = run_bass_kernel_spmd(nc, in_maps, core_ids=list(range(N_CORES)),
                               trace=trace, **(trace_kwargs or {}))
    img_out = np.concatenate([res.results[c]["oimg"] for c in range(N_CORES)], axis=0)
    txt_out = np.concatenate([res.results[c]["otxt"] for c in range(N_CORES)], axis=0)
    return (img_out, txt_out), res


def kernel(image_features, text_features, ln_g, ln_b, Wq, bq, Wk, bk, Wv, bv):
    (img_out, txt_out), _ = run(image_features, text_features, ln_g, ln_b,
                                Wq, bq, Wk, bk, Wv, bv)
    return (img_out, txt_out)
